# revision 1
# baseline (speedup 1.0000x reference)
"""Causal self-attention (B=2, T=2048, C=1024, H=16) on 8 trn2 NeuronCores.

Sharding: batch x head-group. Core c handles batch c//4 and the 4 heads
[4*(c%4), 4*(c%4)+4), as two head-pairs A=(h0,h1), B=(h2,h3). Each core
reads only its batch's half of x (8MB instead of 16MB) and writes an
8MB partial output; the host sums 4 partials per batch and adds b_proj.

Per core:
  - QKV projection of its batch (6 groups of 128 weight cols: qA kA vA
    qB kB vB), producing qT/kT in [head_dim, T] layout and V' in
    [T, head_dim] layout via PE transposes, with a ones column per head
    (softmax denominator accumulates during att@V).
  - Flash-style causal attention per (head-pair, q-block): concurrent
    row-group S^T matmul pairs into a 2-bank PSUM tile, one ACT exp for
    both heads, triangular 0/1 mask on diagonal tiles (GpSimd), att@V
    accumulates per head with the denominator in PSUM row 64.
  - Softmax normalization fully on-chip (no DRAM bounce) and off the
    critical path: po PSUM banks are released immediately (denominator
    rows -> SBUF partitions 0/32 on ACT, un-normalized y -> ytb on DVE);
    one block later a single K=33 selector matmul broadcasts both heads'
    denominators into one PSUM bank, one DVE reciprocal and two in-place
    multiplies normalize ytb before the (two-block-deferred) projection.
  - Output projection contracts 256 y-dims (both head-pairs) per
    128-query chunk; partial [2048, 1024] written to DRAM.
Emission order is tuned for the per-engine FIFO: QKV group chains are
fed between attention k-tiles so ACT's exp stream never starves the PE,
normalization is deferred one block and projection two blocks.

Matmuls run in float32r (fp32 with 12-bit mantissa, 4x faster than fp32
on the PE, full fp32 PSUM accumulate).
"""

import sys

sys.path.insert(0, "/opt/trn_rl_repo")

import numpy as np

B, T, C, H, HD = 2, 2048, 1024, 16, 64
NCORE = 8
HPC = 4           # heads per core
NT = T // 512     # 4 T-tiles (one batch per core)
CCH = C // 128    # 8 contraction chunks
NKT = T // 128    # 16 k-tiles


def _round_f32r(x):
    x = np.ascontiguousarray(x, dtype=np.float32)
    xi = x.view(np.uint32)
    r = (xi + np.uint32(0x7FF) + ((xi >> np.uint32(12)) & np.uint32(1))) & np.uint32(
        0xFFFFF000
    )
    return r.view(np.float32)


_CACHE = {}


def _build():
    if "nc" in _CACHE:
        return _CACHE["nc"]
    from contextlib import ExitStack

    import concourse.bass as bass
    import concourse.bacc as bacc
    import concourse.mybir as mybir
    import concourse.tile as tile
    from concourse.masks import make_identity, make_upper_triangular

    f32, f32r = mybir.dt.float32, mybir.dt.float32r
    AF = mybir.ActivationFunctionType

    nc = bacc.Bacc(None, target_bir_lowering=False, debug=False)
    # x pre-permuted on host to [p, tt, cc, t] so each T-tile DMA reads
    # contiguous runs per partition
    xT_d = nc.dram_tensor("xT", [128, NT, CCH, 512], f32r, kind="ExternalInput")
    wqkv_d = nc.dram_tensor("wqkv", [128, CCH, 6 * 128], f32r, kind="ExternalInput")
    bqkv_d = nc.dram_tensor("bqkv", [128, 6], f32, kind="ExternalInput")
    wp_d = nc.dram_tensor("wp", [128, 2, C], f32r, kind="ExternalInput")
    sel_d = nc.dram_tensor("sel", [33, 128], f32r, kind="ExternalInput")
    out_d = nc.dram_tensor("out", [T, C], f32, kind="ExternalOutput")

    with tile.TileContext(nc) as tc, ExitStack() as ctx:
        sb = ctx.enter_context(tc.tile_pool(name="sb", bufs=1))
        xp = ctx.enter_context(tc.tile_pool(name="xp", bufs=2))
        vtp = ctx.enter_context(tc.tile_pool(name="vtp", bufs=2))
        esp = ctx.enter_context(tc.tile_pool(name="esp", bufs=4))
        dnp = ctx.enter_context(tc.tile_pool(name="dnp", bufs=2))
        rcpp = ctx.enter_context(tc.tile_pool(name="rcpp", bufs=2))
        outp = ctx.enter_context(tc.tile_pool(name="outp", bufs=3))
        # PSUM: 2 + 4 + 2 = 8 banks
        pa = ctx.enter_context(tc.tile_pool(name="pa", bufs=2, space="PSUM"))
        pss = ctx.enter_context(tc.tile_pool(name="pss", bufs=2, space="PSUM"))
        pso = ctx.enter_context(tc.tile_pool(name="pso", bufs=2, space="PSUM"))

        wq_sb = sb.tile([128, CCH, 6 * 128], f32r, tag="wq")

        qT = [sb.tile([128, T], f32r, tag=f"qT{hp}", name=f"qT{hp}") for hp in range(2)]
        kT = [sb.tile([128, T], f32r, tag=f"kT{hp}", name=f"kT{hp}") for hp in range(2)]
        # V' per head-pair: per k-tile [V_h0 (64) | 1 | V_h1 (64) | 1] = 130;
        # the ones column accumulates the softmax denominator during att@v
        VW = 130
        Vp = [
            sb.tile([128, NKT, VW], f32r, tag=f"Vp{hp}", name=f"Vp{hp}")
            for hp in range(2)
        ]
        for hp in range(2):
            nc.vector.memset(Vp[hp][:, :, :].bitcast(f32), 1.0)
        # normalized attention outputs, persistent across the two passes
        ytb = [
            [
                sb.tile([128, 512], f32r, tag=f"ytb{hp}{qb}", name=f"ytb{hp}{qb}")
                for qb in range(NT)
            ]
            for hp in range(2)
        ]
        # selector [33, 128]: col j reads partition 0 (head0 denom) for j<64,
        # partition 32 (head1 denom) for j>=64 -- one K=33 matmul broadcasts
        # both heads' denominator rows into a single [128, 512] PSUM bank
        # (32-aligned partition bases are an ISA requirement)
        sel_sb = sb.tile([33, 128], f32r, tag="sel")
        nc.sync.dma_start(out=sel_sb, in_=sel_d[:, :])

        # ---------------- QKV projection ----------------
        x_tiles = [None] * NT

        def emit_x_dma(tt):
            x_t = xp.tile([128, CCH, 512], f32r, tag="x", name=f"x{tt}")
            if tt == 0:
                # chunked loads so the first matmul chain starts early
                for cc in range(CCH):
                    nc.sync.dma_start(out=wq_sb[:, cc, :], in_=wqkv_d[:, cc, :])
                    nc.sync.dma_start(out=x_t[:, cc, :], in_=xT_d[:, 0, cc, :])
            else:
                nc.sync.dma_start(out=x_t, in_=xT_d[:, tt, :, :])
            x_tiles[tt] = x_t

        def emit_qkv_group(tt, g):
            # g: 0=qA 1=kA 2=vA 3=qB 4=kB 5=vB
            hp, kind = divmod(g, 3)
            x_t = x_tiles[tt]
            tloc = tt * 512
            ps = pa.tile([128, 512], f32, tag="mm")
            for cc in range(CCH):
                nc.tensor.matmul(
                    ps,
                    wq_sb[:, cc, g * 128 : (g + 1) * 128],
                    x_t[:, cc, :],
                    start=(cc == 0),
                    stop=(cc == CCH - 1),
                )
            if kind == 0:
                nc.vector.tensor_scalar_add(
                    qT[hp][:, tloc : tloc + 512], ps, bias_sb[:, g : g + 1]
                )
            elif kind == 1:
                nc.vector.tensor_scalar_add(
                    kT[hp][:, tloc : tloc + 512], ps, bias_sb[:, g : g + 1]
                )
            else:
                v_t = vtp.tile([128, 512], f32, tag="v")
                nc.vector.tensor_scalar_add(v_t, ps, bias_sb[:, g : g + 1])
                for j in range(4):
                    pt = pa.tile([128, 128], f32, tag="mm")
                    nc.tensor.transpose(pt, v_t[:, j * 128 : (j + 1) * 128], ident)
                    ktl = tt * 4 + j
                    # one strided copy moves both heads' V columns
                    nc.vector.tensor_copy(
                        Vp[hp][:, ktl, 0:130].rearrange("p (s e) -> p s e", s=2)[
                            :, :, 0:64
                        ],
                        pt[:, :].rearrange("p (s e) -> p s e", s=2),
                    )

        emit_x_dma(0)
        # constants not needed immediately: emit loads after the x chunks
        bias_sb = sb.tile([128, 6], f32, tag="bias")
        nc.sync.dma_start(out=bias_sb, in_=bqkv_d[:, :])
        emit_x_dma(1)
        wp_sb = sb.tile([128, 2, C], f32r, tag="wp")
        nc.sync.dma_start(out=wp_sb, in_=wp_d[:, :, :])
        ident = sb.tile([128, 128], f32, tag="ident")
        make_identity(nc, ident)
        tri2 = sb.tile([128, 2, 128], f32, tag="tri2")
        make_upper_triangular(nc, tri2[:, 0, :], val=1.0, diag=True)
        nc.gpsimd.tensor_copy(tri2[:, 1, :], tri2[:, 0, :])
        # PE warm-up: junk matmuls during the initial DMA wait pull the HAM
        # clock gate to 8/8 before the first real qkv chain issues
        wu = pa.tile([128, 128], f32, tag="mm", name="warmup")
        for _ in range(16):
            nc.tensor.matmul(wu, ident, ident, start=True, stop=True)
        for g in range(6):
            emit_qkv_group(0, g)

        # ------------- attention + normalization + projection -------------
        scale = 1.0 / 8.0  # 1/sqrt(HD)
        deferred_norm = []
        proj_queue = []

        def make_norm(dh, hp, qb):
            def norm():
                # broadcast both heads' denominators into one PSUM bank via a
                # single K=2 matmul (selector stationary), one reciprocal,
                # then normalize ytb in place. Runs lazily one block later --
                # nothing here holds PSUM po banks.
                den = pa.tile([128, 512], f32, tag="mm", name="den")
                nc.tensor.matmul(den, sel_sb[0:33, :], dh[0:33, :], start=True, stop=True)
                rcp = rcpp.tile([128, 512], f32, tag="rcp")
                nc.vector.reciprocal(rcp, den)
                yt = ytb[hp][qb]
                nc.vector.tensor_mul(yt[0:64, :], yt[0:64, :], rcp[0:64, :])
                nc.vector.tensor_mul(yt[64:128, :], yt[64:128, :], rcp[64:128, :])
            return norm

        def emit_proj(qb):
            for j in range(4):
                out_t = outp.tile([128, C], f32, tag="out", name="out_t")
                js = slice(j * 128, (j + 1) * 128)
                for ncol in range(2):
                    cs = slice(ncol * 512, (ncol + 1) * 512)
                    pp = pa.tile([128, 512], f32, tag="mm", name="pp")
                    nc.tensor.matmul(
                        pp, ytb[0][qb][:, js], wp_sb[:, 0, cs], start=True, stop=False
                    )
                    nc.tensor.matmul(
                        pp, ytb[1][qb][:, js], wp_sb[:, 1, cs], start=False, stop=True
                    )
                    if ncol == 0:
                        nc.scalar.copy(out_t[:, cs], pp)
                    else:
                        nc.vector.tensor_copy(out_t[:, cs], pp)
                row = qb * 512 + j * 128
                nc.sync.dma_start(out=out_d[row : row + 128, :], in_=out_t)

        def emit_att_block(hp, qb, feeds=()):
            feeds = list(feeds)
            n_kt = 4 * (qb + 1)
            po = [
                pso.tile([65, 512], f32, tag="po", name=f"po{hp}{qb}{h}")
                for h in range(2)
            ]
            pend = []  # att@v pipelined two k-tiles behind S/exp
            for lkt in range(n_kt):
                r0 = max(0, (lkt - 4 * qb) * 128)
                ks = slice(lkt * 128, (lkt + 1) * 128)
                qs = slice(qb * 512 + r0, (qb + 1) * 512)
                ps2 = pss.tile([128, 1024], f32, tag="s2")
                nc.tensor.matmul(
                    ps2[:, r0:512], kT[hp][0:64, ks], qT[hp][0:64, qs],
                    start=True, stop=True,
                )
                nc.tensor.matmul(
                    ps2[:, 512 + r0 : 1024], kT[hp][64:128, ks], qT[hp][64:128, qs],
                    start=True, stop=True,
                )
                es = esp.tile([128, 1024], f32r, tag="es")
                if r0:
                    nc.scalar.activation(
                        es[:, :].rearrange("p (h q) -> p h q", h=2)[:, :, r0:512],
                        ps2[:, :].rearrange("p (h q) -> p h q", h=2)[:, :, r0:512],
                        AF.Exp,
                        scale=scale,
                    )
                else:
                    nc.scalar.activation(es, ps2, AF.Exp, scale=scale)
                if lkt >= 4 * qb:  # diagonal tile: causal mask, both heads
                    nc.gpsimd.tensor_mul(
                        es[:, :].rearrange("p (h q) -> p h q", h=2)[
                            :, :, r0 : r0 + 128
                        ],
                        es[:, :].rearrange("p (h q) -> p h q", h=2)[
                            :, :, r0 : r0 + 128
                        ],
                        tri2[:, :, :],
                    )
                if lkt == 0 and deferred_norm:
                    deferred_norm.pop()()
                if lkt == 1 and len(proj_queue) >= 2:
                    emit_proj(proj_queue.pop(0))
                if feeds:
                    feeds.pop(0)()
                if len(pend) >= 2:
                    for mm in pend.pop(0):
                        nc.tensor.matmul(**mm)
                pend.append(
                    [
                        dict(
                            out=po[h][:, r0:512],
                            lhsT=Vp[hp][:, lkt, h * 65 : (h + 1) * 65],
                            rhs=es[:, h * 512 + r0 : (h + 1) * 512],
                            start=(lkt == 0),
                            stop=(lkt == n_kt - 1),
                        )
                        for h in range(2)
                    ]
                )
            for grp in pend:
                for mm in grp:
                    nc.tensor.matmul(**mm)
            for f in feeds:  # leftover feeds (short blocks)
                f()
            # evacuate po fast so the next block's att@v gets its PSUM banks:
            # denominator rows -> SBUF partitions 0/1 on ACT, un-normalized y
            # -> ytb on DVE (reciprocal + normalize run lazily, a block later)
            dh = dnp.tile([33, 512], f32r, tag="dh", name="dh")
            yt = ytb[hp][qb]
            nc.scalar.copy(dh[0:1, :], po[0][64:65, :])
            nc.vector.tensor_copy(yt[0:64, :], po[0][0:64, :])
            nc.scalar.copy(dh[32:33, :], po[1][64:65, :])
            nc.vector.tensor_copy(yt[64:128, :], po[1][0:64, :])
            deferred_norm.append(make_norm(dh, hp, qb))

        # Schedule: pass A ascending (attention starts right after the first
        # qkv tile; later qkv tiles are fed between its k-tiles), pass B
        # descending (tail block is the smallest). Projections are deferred
        # two blocks so their PE ops never wait on fresh normalizations.
        def feed_funcs(tt, with_dma):
            fs = []
            if with_dma:
                fs.append(lambda tt=tt: emit_x_dma(tt))
            for g in range(6):
                fs.append(lambda tt=tt, g=g: emit_qkv_group(tt, g))
            return fs

        emit_att_block(0, 0, feeds=feed_funcs(1, False) + [lambda: emit_x_dma(2)])
        emit_att_block(0, 1, feeds=[lambda: emit_x_dma(3)] + feed_funcs(2, False))
        emit_att_block(0, 2, feeds=feed_funcs(3, False))
        emit_att_block(0, 3)
        for qb in (3, 2, 1, 0):
            emit_att_block(1, qb)
            proj_queue.append(qb)
        while deferred_norm:
            deferred_norm.pop()()
        while proj_queue:
            emit_proj(proj_queue.pop(0))

    nc.finalize()
    _CACHE["nc"] = nc
    return nc


def _prep_inputs(x, w_attn, b_attn, w_proj):
    x = np.ascontiguousarray(np.asarray(x, dtype=np.float32))
    w_attn = np.asarray(w_attn, dtype=np.float32)
    b_attn = np.asarray(b_attn, dtype=np.float32)
    w_proj = np.asarray(w_proj, dtype=np.float32)

    # per batch: xT[p, tt, cc, t] = x[b, tt*512+t, cc*128+p]
    xTs = [
        _round_f32r(x[b].reshape(NT, 512, CCH, 128).transpose(3, 0, 2, 1))
        for b in range(B)
    ]
    in_maps = []
    for c in range(NCORE):
        b = c // 4
        hq = (c % 4) * HPC  # first global head on this core
        blocks = []
        bias_cols = []
        for hp in range(2):
            hs = [hq + 2 * hp, hq + 2 * hp + 1]
            for off in (0, C, 2 * C):  # q, k, v
                for h in hs:
                    blocks.append(w_attn[:, off + h * HD : off + (h + 1) * HD])
                bias_cols.append(
                    np.concatenate(
                        [b_attn[off + h * HD : off + (h + 1) * HD] for h in hs]
                    )
                )
        wq_flat = _round_f32r(np.concatenate(blocks, axis=1))  # [C, 768]
        wqkv = np.ascontiguousarray(
            wq_flat.reshape(CCH, 128, 6 * 128).transpose(1, 0, 2)
        )
        bqkv = np.ascontiguousarray(np.stack(bias_cols, axis=1))  # [128, 6]
        wp = _round_f32r(
            w_proj[hq * HD : hq * HD + 256, :].reshape(2, 128, C).transpose(1, 0, 2)
        )  # [128, 2, C]
        sel = np.zeros((33, 128), dtype=np.float32)
        sel[0, 0:64] = 1.0
        sel[32, 64:128] = 1.0
        in_maps.append(
            {"xT": xTs[b], "wqkv": wqkv, "bqkv": bqkv, "wp": wp, "sel": sel}
        )
    return in_maps


def _run(x, w_attn, b_attn, w_proj, b_proj, trace=False, tmpdir=None):
    from concourse.bass_utils import run_bass_kernel_spmd

    nc = _build()
    in_maps = _prep_inputs(x, w_attn, b_attn, w_proj)
    res = run_bass_kernel_spmd(
        nc, in_maps, list(range(NCORE)), trace=trace, tmpdir=tmpdir
    )
    bp = np.asarray(b_proj, dtype=np.float64)
    outs = []
    for b in range(B):
        acc = np.sum(
            np.stack([res.results[b * 4 + i]["out"] for i in range(4)]),
            axis=0,
            dtype=np.float64,
        )
        outs.append((acc + bp).astype(np.float32))
    return np.stack(outs), res


def kernel(x, w_attn, b_attn, w_proj, b_proj):
    out, _ = _run(x, w_attn, b_attn, w_proj, b_proj, trace=False)
    return out



# revision 19
# speedup vs baseline: 1.3160x; 1.3160x over previous
"""Causal self-attention (B=2, T=2048, C=1024, H=16) on 8 trn2 NeuronCores.

Sharding: batch x head-group. Core c handles batch c//4 and the 4 heads
[4*(c%4), 4*(c%4)+4), as two head-pairs A=(h0,h1), B=(h2,h3). Each core
reads only its batch's half of x (8MB instead of 16MB) and writes an
8MB partial output; the host sums 4 partials per batch and adds b_proj.

Per core:
  - QKV projection of its batch (6 groups of 128 weight cols: qA kA vA
    qB kB vB), producing qT/kT in [head_dim, T] layout and V' in
    [T, head_dim] layout via PE transposes, with a ones column per head
    (softmax denominator accumulates during att@V).
  - Flash-style causal attention per (head-pair, q-block): concurrent
    row-group S^T matmul pairs into a 2-bank PSUM tile, one ACT exp for
    both heads, triangular 0/1 mask on diagonal tiles (GpSimd), att@V
    accumulates per head with the denominator in PSUM row 64.
  - Softmax normalization fully on-chip (no DRAM bounce) and off the
    critical path: po PSUM banks are released immediately (denominator
    rows -> SBUF partitions 0/32 on ACT, un-normalized y -> ytb on DVE);
    one block later a single K=33 selector matmul broadcasts both heads'
    denominators into one PSUM bank, one DVE reciprocal and two in-place
    multiplies normalize ytb before the (two-block-deferred) projection.
  - Output projection contracts 256 y-dims (both head-pairs) per
    128-query chunk; partial [2048, 1024] written to DRAM.
Emission order is tuned for the per-engine FIFO: QKV group chains are
fed between attention k-tiles so ACT's exp stream never starves the PE,
normalization is deferred one block and projection two blocks.

Matmuls run in bfloat16 (same PE column throughput as f32r, but half
the DMA/SBUF traffic and lower power -> less HAM/P0 throttling), with
full fp32 PSUM accumulate. rel err ~3e-3 vs the 2e-2 gate.
"""

import sys

sys.path.insert(0, "/opt/trn_rl_repo")

import numpy as np

B, T, C, H, HD = 2, 2048, 1024, 16, 64
NCORE = 8
HPC = 4           # heads per core
NT = T // 512     # 4 T-tiles (one batch per core)
CCH = C // 128    # 8 contraction chunks
NKT = T // 128    # 16 k-tiles


def _to_bf16(x):
    import ml_dtypes

    return np.ascontiguousarray(np.asarray(x, dtype=np.float32)).astype(
        ml_dtypes.bfloat16
    )


_CACHE = {}


def _build():
    if "nc" in _CACHE:
        return _CACHE["nc"]
    from contextlib import ExitStack

    import concourse.bass as bass
    import concourse.bacc as bacc
    import concourse.mybir as mybir
    import concourse.tile as tile
    from concourse.masks import make_identity, make_upper_triangular

    f32, bf16 = mybir.dt.float32, mybir.dt.bfloat16
    AF = mybir.ActivationFunctionType

    nc = bacc.Bacc(None, target_bir_lowering=False, debug=False)
    # x pre-permuted on host to [p, tt, cc, t] so each T-tile DMA reads
    # contiguous runs per partition
    xT_d = nc.dram_tensor("xT", [128, NT, CCH, 512], bf16, kind="ExternalInput")
    wqkv_d = nc.dram_tensor("wqkv", [128, CCH, 6 * 128], bf16, kind="ExternalInput")
    bqkv_d = nc.dram_tensor("bqkv", [128, 6], f32, kind="ExternalInput")
    wp_d = nc.dram_tensor("wp", [128, 2, C], bf16, kind="ExternalInput")
    sel_d = nc.dram_tensor("sel", [33, 128], bf16, kind="ExternalInput")
    out_d = nc.dram_tensor("out", [T, C], f32, kind="ExternalOutput")

    with tile.TileContext(nc) as tc, ExitStack() as ctx:
        sb = ctx.enter_context(tc.tile_pool(name="sb", bufs=1))
        xp = ctx.enter_context(tc.tile_pool(name="xp", bufs=2))
        vtp = ctx.enter_context(tc.tile_pool(name="vtp", bufs=2))
        esp = ctx.enter_context(tc.tile_pool(name="esp", bufs=4))
        rcpp = ctx.enter_context(tc.tile_pool(name="rcpp", bufs=2))
        outp = ctx.enter_context(tc.tile_pool(name="outp", bufs=3))
        # PSUM: 2 + 4 + 2 = 8 banks
        pa = ctx.enter_context(tc.tile_pool(name="pa", bufs=2, space="PSUM"))
        pss = ctx.enter_context(tc.tile_pool(name="pss", bufs=2, space="PSUM"))
        pso = ctx.enter_context(tc.tile_pool(name="pso", bufs=2, space="PSUM"))

        wq_sb = sb.tile([128, CCH, 6 * 128], bf16, tag="wq")

        qT = [sb.tile([128, T], bf16, tag=f"qT{hp}", name=f"qT{hp}") for hp in range(2)]
        kT = [sb.tile([128, T], bf16, tag=f"kT{hp}", name=f"kT{hp}") for hp in range(2)]
        # V' per head-pair: per k-tile [V_h0 (64) | 1 | V_h1 (64) | 1] = 130;
        # the ones column accumulates the softmax denominator during att@v
        VW = 130
        Vp = [
            sb.tile([128, NKT, VW], bf16, tag=f"Vp{hp}", name=f"Vp{hp}")
            for hp in range(2)
        ]
        for hp in range(2):
            nc.vector.memset(Vp[hp][:, :, :], 1.0)
        # normalized attention outputs, persistent across the two passes
        ytb = [
            [
                sb.tile([128, 512], bf16, tag=f"ytb{hp}{qb}", name=f"ytb{hp}{qb}")
                for qb in range(NT)
            ]
            for hp in range(2)
        ]
        # selector [33, 128]: col j reads partition 0 (head0 denom) for j<64,
        # partition 32 (head1 denom) for j>=64 -- one K=33 matmul broadcasts
        # both heads' denominator rows into a single [128, 512] PSUM bank
        # (32-aligned partition bases are an ISA requirement)
        sel_sb = sb.tile([33, 128], bf16, tag="sel")
        nc.sync.dma_start(out=sel_sb, in_=sel_d[:, :])
        # persistent double-buffered denominator tiles; rows 1..31/33.. are
        # never written and must be finite (the selector matmul touches all
        # 33 partitions), so memset the whole tiles once
        dh_tiles = [
            sb.tile([33, 512], bf16, tag=f"dh{i}", name=f"dh{i}") for i in range(2)
        ]
        for t in dh_tiles:
            nc.vector.memset(t[:, :], 1.0)
        dh_idx = [0]

        # ---------------- QKV projection ----------------
        x_tiles = [None] * NT

        def emit_x_dma(tt):
            x_t = xp.tile([128, CCH, 512], bf16, tag="x", name=f"x{tt}")
            if tt == 0:
                # chunked loads so the first matmul chain starts early
                for cc in range(CCH):
                    nc.sync.dma_start(out=wq_sb[:, cc, :], in_=wqkv_d[:, cc, :])
                    nc.sync.dma_start(out=x_t[:, cc, :], in_=xT_d[:, 0, cc, :])
            else:
                nc.sync.dma_start(out=x_t, in_=xT_d[:, tt, :, :])
            x_tiles[tt] = x_t

        def emit_qkv_group(tt, g):
            # g: 0=qA 1=kA 2=vA 3=qB 4=kB 5=vB
            hp, kind = divmod(g, 3)
            x_t = x_tiles[tt]
            tloc = tt * 512
            ps = pa.tile([128, 512], f32, tag="mm")
            for cc in range(CCH):
                nc.tensor.matmul(
                    ps,
                    wq_sb[:, cc, g * 128 : (g + 1) * 128],
                    x_t[:, cc, :],
                    start=(cc == 0),
                    stop=(cc == CCH - 1),
                )
            if kind == 0:
                nc.vector.tensor_scalar_add(
                    qT[hp][:, tloc : tloc + 512], ps, bias_sb[:, g : g + 1]
                )
            elif kind == 1:
                nc.vector.tensor_scalar_add(
                    kT[hp][:, tloc : tloc + 512], ps, bias_sb[:, g : g + 1]
                )
            else:
                v_t = vtp.tile([128, 512], bf16, tag="v")
                nc.vector.tensor_scalar_add(v_t, ps, bias_sb[:, g : g + 1])
                for j in range(4):
                    pt = pa.tile([128, 128], bf16, tag="mm")
                    nc.tensor.transpose(pt, v_t[:, j * 128 : (j + 1) * 128], ident)
                    ktl = tt * 4 + j
                    # one strided copy moves both heads' V columns
                    nc.vector.tensor_copy(
                        Vp[hp][:, ktl, 0:130].rearrange("p (s e) -> p s e", s=2)[
                            :, :, 0:64
                        ],
                        pt[:, :].rearrange("p (s e) -> p s e", s=2),
                    )

        emit_x_dma(0)
        # constants not needed immediately: emit loads after the x chunks
        bias_sb = sb.tile([128, 6], f32, tag="bias")
        nc.sync.dma_start(out=bias_sb, in_=bqkv_d[:, :])
        emit_x_dma(1)
        wp_sb = sb.tile([128, 2, C], bf16, tag="wp")
        nc.sync.dma_start(out=wp_sb, in_=wp_d[:, :, :])
        ident = sb.tile([128, 128], bf16, tag="ident")
        make_identity(nc, ident)
        tri2 = sb.tile([128, 2, 128], bf16, tag="tri2")
        make_upper_triangular(nc, tri2[:, 0, :], val=1.0, diag=True)
        nc.gpsimd.tensor_copy(tri2[:, 1, :], tri2[:, 0, :])
        # PE warm-up: junk matmuls during the initial DMA wait pull the HAM
        # clock gate to 8/8 before the first real qkv chain issues
        wu = pa.tile([128, 128], f32, tag="mm", name="warmup")
        for _ in range(16):
            nc.tensor.matmul(wu, ident, ident, start=True, stop=True)
        for g in range(6):
            emit_qkv_group(0, g)

        # ------------- attention + normalization + projection -------------
        scale = 1.0 / 8.0  # 1/sqrt(HD)
        deferred_norm = []
        proj_queue = []

        def make_norm(dh, hp, qb):
            def norm():
                # broadcast both heads' denominators into one PSUM bank via a
                # single K=2 matmul (selector stationary), one reciprocal,
                # then normalize ytb in place. Runs lazily one block later --
                # nothing here holds PSUM po banks.
                den = pa.tile([128, 512], f32, tag="mm", name="den")
                nc.tensor.matmul(den, sel_sb[0:33, :], dh[0:33, :], start=True, stop=True)
                rcp = rcpp.tile([128, 512], f32, tag="rcp")
                nc.vector.reciprocal_approx_fast(out=rcp, in_=den)
                yt = ytb[hp][qb]
                nc.vector.tensor_mul(yt[0:64, :], yt[0:64, :], rcp[0:64, :])
                nc.vector.tensor_mul(yt[64:128, :], yt[64:128, :], rcp[64:128, :])
            return norm

        def emit_proj(qb):
            for j in range(4):
                out_t = outp.tile([128, C], f32, tag="out", name="out_t")
                js = slice(j * 128, (j + 1) * 128)
                for ncol in range(2):
                    cs = slice(ncol * 512, (ncol + 1) * 512)
                    pp = pa.tile([128, 512], f32, tag="mm", name="pp")
                    nc.tensor.matmul(
                        pp, ytb[0][qb][:, js], wp_sb[:, 0, cs], start=True, stop=False
                    )
                    nc.tensor.matmul(
                        pp, ytb[1][qb][:, js], wp_sb[:, 1, cs], start=False, stop=True
                    )
                    nc.vector.tensor_copy(out_t[:, cs], pp)
                row = qb * 512 + j * 128
                nc.sync.dma_start(out=out_d[row : row + 128, :], in_=out_t)

        def emit_att_block(hp, qb, feeds=()):
            feeds = list(feeds)
            n_kt = 4 * (qb + 1)
            po = [
                pso.tile([65, 512], f32, tag="po", name=f"po{hp}{qb}{h}")
                for h in range(2)
            ]
            pend = []  # att@v pipelined two k-tiles behind S/exp
            for lkt in range(n_kt):
                r0 = max(0, (lkt - 4 * qb) * 128)
                ks = slice(lkt * 128, (lkt + 1) * 128)
                qs = slice(qb * 512 + r0, (qb + 1) * 512)
                ps2 = pss.tile([128, 1024], f32, tag="s2")
                nc.tensor.matmul(
                    ps2[:, r0:512], kT[hp][0:64, ks], qT[hp][0:64, qs],
                    start=True, stop=True,
                )
                nc.tensor.matmul(
                    ps2[:, 512 + r0 : 1024], kT[hp][64:128, ks], qT[hp][64:128, qs],
                    start=True, stop=True,
                )
                es = esp.tile([128, 1024], bf16, tag="es")
                if r0:
                    nc.scalar.activation(
                        es[:, :].rearrange("p (h q) -> p h q", h=2)[:, :, r0:512],
                        ps2[:, :].rearrange("p (h q) -> p h q", h=2)[:, :, r0:512],
                        AF.Exp,
                        scale=scale,
                    )
                else:
                    nc.scalar.activation(es, ps2, AF.Exp, scale=scale)
                if lkt >= 4 * qb:  # diagonal tile: causal mask, both heads
                    nc.gpsimd.tensor_mul(
                        es[:, :].rearrange("p (h q) -> p h q", h=2)[
                            :, :, r0 : r0 + 128
                        ],
                        es[:, :].rearrange("p (h q) -> p h q", h=2)[
                            :, :, r0 : r0 + 128
                        ],
                        tri2[:, :, :],
                    )
                if lkt == 0 and deferred_norm:
                    deferred_norm.pop()()
                if lkt == 1 and len(proj_queue) >= 2:
                    emit_proj(proj_queue.pop(0))
                if feeds:
                    feeds.pop(0)()
                if len(pend) >= 2:
                    for mm in pend.pop(0):
                        nc.tensor.matmul(**mm)
                pend.append(
                    [
                        dict(
                            out=po[h][:, r0:512],
                            lhsT=Vp[hp][:, lkt, h * 65 : (h + 1) * 65],
                            rhs=es[:, h * 512 + r0 : (h + 1) * 512],
                            start=(lkt == 0),
                            stop=(lkt == n_kt - 1),
                        )
                        for h in range(2)
                    ]
                )
            for grp in pend:
                for mm in grp:
                    nc.tensor.matmul(**mm)
            for f in feeds:  # leftover feeds (short blocks)
                f()
            # evacuate po fast so the next block's att@v gets its PSUM banks:
            # denominator rows -> SBUF partitions 0/1 on ACT, un-normalized y
            # -> ytb on DVE (reciprocal + normalize run lazily, a block later)
            dh = dh_tiles[dh_idx[0]]
            dh_idx[0] ^= 1
            yt = ytb[hp][qb]
            nc.scalar.copy(dh[0:1, :], po[0][64:65, :])
            nc.vector.tensor_copy(yt[0:64, :], po[0][0:64, :])
            nc.scalar.copy(dh[32:33, :], po[1][64:65, :])
            nc.vector.tensor_copy(yt[64:128, :], po[1][0:64, :])
            deferred_norm.append(make_norm(dh, hp, qb))

        # Schedule: pass A ascending (attention starts right after the first
        # qkv tile; later qkv tiles are fed between its k-tiles), pass B
        # descending (tail block is the smallest). Projections are deferred
        # two blocks so their PE ops never wait on fresh normalizations.
        def feed_funcs(tt, with_dma):
            fs = []
            if with_dma:
                fs.append(lambda tt=tt: emit_x_dma(tt))
            for g in range(6):
                fs.append(lambda tt=tt, g=g: emit_qkv_group(tt, g))
            return fs

        emit_att_block(0, 0, feeds=feed_funcs(1, False) + [lambda: emit_x_dma(2)])
        emit_att_block(0, 1, feeds=[lambda: emit_x_dma(3)] + feed_funcs(2, False))
        emit_att_block(0, 2, feeds=feed_funcs(3, False))
        emit_att_block(0, 3)
        for qb in (3, 2, 1, 0):
            emit_att_block(1, qb)
            proj_queue.append(qb)
        while deferred_norm:
            deferred_norm.pop()()
        while proj_queue:
            emit_proj(proj_queue.pop(0))

    nc.finalize()
    _CACHE["nc"] = nc
    return nc


def _prep_inputs(x, w_attn, b_attn, w_proj):
    x = np.ascontiguousarray(np.asarray(x, dtype=np.float32))
    w_attn = np.asarray(w_attn, dtype=np.float32)
    b_attn = np.asarray(b_attn, dtype=np.float32)
    w_proj = np.asarray(w_proj, dtype=np.float32)

    # per batch: xT[p, tt, cc, t] = x[b, tt*512+t, cc*128+p]
    xTs = [
        _to_bf16(x[b].reshape(NT, 512, CCH, 128).transpose(3, 0, 2, 1))
        for b in range(B)
    ]
    in_maps = []
    for c in range(NCORE):
        b = c // 4
        hq = (c % 4) * HPC  # first global head on this core
        blocks = []
        bias_cols = []
        for hp in range(2):
            hs = [hq + 2 * hp, hq + 2 * hp + 1]
            for off in (0, C, 2 * C):  # q, k, v
                for h in hs:
                    blocks.append(w_attn[:, off + h * HD : off + (h + 1) * HD])
                bias_cols.append(
                    np.concatenate(
                        [b_attn[off + h * HD : off + (h + 1) * HD] for h in hs]
                    )
                )
        wq_flat = _to_bf16(np.concatenate(blocks, axis=1))  # [C, 768]
        wqkv = np.ascontiguousarray(
            wq_flat.reshape(CCH, 128, 6 * 128).transpose(1, 0, 2)
        )
        bqkv = np.ascontiguousarray(
            np.stack(bias_cols, axis=1).astype(np.float32)
        )  # [128, 6]
        wp = _to_bf16(
            w_proj[hq * HD : hq * HD + 256, :].reshape(2, 128, C).transpose(1, 0, 2)
        )  # [128, 2, C]
        sel = np.zeros((33, 128), dtype=np.float32)
        sel[0, 0:64] = 1.0
        sel[32, 64:128] = 1.0
        sel = _to_bf16(sel)
        in_maps.append(
            {"xT": xTs[b], "wqkv": wqkv, "bqkv": bqkv, "wp": wp, "sel": sel}
        )
    return in_maps


def _run(x, w_attn, b_attn, w_proj, b_proj, trace=False, tmpdir=None):
    from concourse.bass_utils import run_bass_kernel_spmd

    nc = _build()
    in_maps = _prep_inputs(x, w_attn, b_attn, w_proj)
    res = run_bass_kernel_spmd(
        nc, in_maps, list(range(NCORE)), trace=trace, tmpdir=tmpdir
    )
    bp = np.asarray(b_proj, dtype=np.float64)
    outs = []
    for b in range(B):
        acc = np.sum(
            np.stack([res.results[b * 4 + i]["out"] for i in range(4)]),
            axis=0,
            dtype=np.float64,
        )
        outs.append((acc + bp).astype(np.float32))
    return np.stack(outs), res


def kernel(x, w_attn, b_attn, w_proj, b_proj):
    out, _ = _run(x, w_attn, b_attn, w_proj, b_proj, trace=False)
    return out

